# revision 5
# baseline (speedup 1.0000x reference)
"""Two-layer GCN (PyG GCNConv style) on 8 Trainium2 NeuronCores.

Strategy (dst-sharded, gather-table formulation):
  h1 = relu(Ahat @ (x @ W1) + b1);  h2 = relu(Ahat @ (h1 @ W2) + b2);  out = h2 @ Wfc + bfc
  with Ahat = D^-1/2 (A + I) D^-1/2.

  Per core (owns 1/8 of destination nodes):
    P0: p = dis[n] * (x[n] @ W1)  for own nodes -> bf16 row table, AllGather -> p_table [NPAD,128]
    P1: Z1[ch,d] = sum_e dis[dst_e] * p_table[src_e, ch]   (edges sorted by (src-chunk, dst-block);
        dma_gather 256B rows; one-hot scatter matmuls accumulate per dst-block in PSUM)
        h1 = relu(Z1 + b1)
    P2: g = dis[n] * (h1[n] @ W2) -> bf16 table, AllGather -> g_table
    P3: same aggregation with g_table -> Z2; h2 = relu(Z2 + b2)
    P4: outT = Wfc^T @ h2 + bfc -> [4, own nodes] fp32

  Host does only index/layout prep: self-loops, degrees->dis, edge bucketing by
  (core, src-chunk, dst-block), padding, int16 index rebasing, and final unshard.
"""

import sys

sys.path.insert(0, "/opt/trn_rl_repo")

from contextlib import ExitStack
from dataclasses import dataclass

import numpy as np
import ml_dtypes

import concourse.bacc as bacc
import concourse.tile as tile
import concourse.mybir as mybir
from concourse.bass_utils import run_bass_kernel_spmd
from concourse.library_config import mlp

F32 = mybir.dt.float32
BF16 = mybir.dt.bfloat16
I16 = mybir.dt.int16


@dataclass(frozen=True)
class Cfg:
    n: int = 100000       # real nodes
    nc: int = 8           # cores
    blk: int = 128
    bpc: int = 98         # blocks per core -> npad = nc*bpc*blk = 100352
    nchunks: int = 4      # int16 index chunks
    call_tiles: int = 8   # tiles (128 idxs each) per dma_gather call (>1024 idxs/call crashes SWDGE)

    @property
    def npad(self):
        return self.nc * self.bpc * self.blk

    @property
    def nodes_pc(self):
        return self.bpc * self.blk

    @property
    def chunk_rows(self):
        return self.npad // self.nchunks


CFG = Cfg()


def _prep(cfg: Cfg, edge_index: np.ndarray):
    """Host-side index prep. Returns (T, meta, per-core arrays, dis)."""
    n, npad = cfg.n, cfg.npad
    src = np.asarray(edge_index[0]).astype(np.int64)
    dst = np.asarray(edge_index[1]).astype(np.int64)
    loops = np.arange(n, dtype=np.int64)
    s = np.concatenate([src, loops])
    d = np.concatenate([dst, loops])

    deg = np.bincount(d, minlength=n).astype(np.float64)
    dis = np.zeros(npad, np.float32)
    dis[:n] = (1.0 / np.sqrt(np.maximum(deg, 1.0))).astype(np.float32)

    core = d // cfg.nodes_pc
    block = (d % cfg.nodes_pc) // cfg.blk
    chunk = s // cfg.chunk_rows
    nkeys = cfg.nc * cfg.nchunks * cfg.bpc
    key = (core * cfg.nchunks + chunk) * cfg.bpc + block
    order = np.argsort(key * (1 << 18) + s, kind="stable")
    s, d, key = s[order], d[order], key[order]

    counts = np.bincount(key, minlength=nkeys)
    tiles = -(-counts.reshape(cfg.nc, cfg.nchunks, cfg.bpc) // cfg.blk)
    T = tiles.max(axis=0)  # [nchunks, bpc] shared across cores
    T[0] = np.maximum(T[0], 1)
    TT = int(T.sum())

    slot_off = np.zeros((cfg.nchunks, cfg.bpc), np.int64)
    off = 0
    for c in range(cfg.nchunks):
        for b in range(cfg.bpc):
            slot_off[c, b] = off
            off += T[c, b] * cfg.blk
    total_slots = off
    assert total_slots == TT * cfg.blk

    idx16 = np.zeros((cfg.nc, total_slots), np.int16)
    dstloc = np.full((cfg.nc, total_slots), 255.0, np.float32)
    ddst = np.zeros((cfg.nc, total_slots), np.float32)

    start = np.zeros(nkeys + 1, np.int64)
    np.cumsum(counts, out=start[1:])
    for co in range(cfg.nc):
        for c in range(cfg.nchunks):
            for b in range(cfg.bpc):
                k = (co * cfg.nchunks + c) * cfg.bpc + b
                lo, hi = start[k], start[k + 1]
                if hi == lo:
                    continue
                sl = slot_off[c, b]
                cnt = hi - lo
                idx16[co, sl : sl + cnt] = (s[lo:hi] - c * cfg.chunk_rows).astype(
                    np.int16
                )
                dstloc[co, sl : sl + cnt] = (
                    d[lo:hi] - (co * cfg.nodes_pc + b * cfg.blk)
                ).astype(np.float32)
                ddst[co, sl : sl + cnt] = dis[d[lo:hi]]

    # wrap idxs: slot i -> partition i%16, col i//16; replicate band to 128 partitions
    idx_wrapped = np.tile(
        idx16.reshape(cfg.nc, total_slots // 16, 16).transpose(0, 2, 1), (1, 8, 1)
    ).copy()  # [nc, 128, total_slots//16]
    # dstc: [nc, 128, TT, 2] (slot i -> tile i//128, lane i%128)
    dl = dstloc.reshape(cfg.nc, TT, cfg.blk).transpose(0, 2, 1)
    dd = ddst.reshape(cfg.nc, TT, cfg.blk).transpose(0, 2, 1)
    dstc = np.stack([dl, dd], axis=-1).reshape(cfg.nc, cfg.blk, TT * 2).copy()

    # call/tile metadata (shared across cores): per chunk, list of calls;
    # each call: (tile_glob_start, [(block, first_in_seg, last_in_seg)])
    meta = []
    gt = 0
    for c in range(cfg.nchunks):
        seq = []
        for b in range(cfg.bpc):
            for j in range(T[c, b]):
                seq.append((b, j == 0, j == T[c, b] - 1))
        calls = []
        for i in range(0, len(seq), cfg.call_tiles):
            calls.append((gt + i, seq[i : i + cfg.call_tiles]))
        meta.append(calls)
        gt += len(seq)

    return T, TT, meta, idx_wrapped, dstc, dis


def _build(cfg: Cfg, TT: int, meta):
    nc = bacc.Bacc("TRN2", target_bir_lowering=False, debug=False, num_devices=cfg.nc)
    npc, blk, bpc = cfg.nodes_pc, cfg.blk, cfg.bpc

    xT_d = nc.dram_tensor("xT", [4, npc], F32, kind="ExternalInput")
    w1_d = nc.dram_tensor("w1", [4, 128], F32, kind="ExternalInput")
    b1_d = nc.dram_tensor("b1", [128, 1], F32, kind="ExternalInput")
    w2_d = nc.dram_tensor("w2", [128, 128], F32, kind="ExternalInput")
    b2_d = nc.dram_tensor("b2", [128, 1], F32, kind="ExternalInput")
    wfc_d = nc.dram_tensor("wfc", [128, 4], F32, kind="ExternalInput")
    bfc_d = nc.dram_tensor("bfc", [4, 1], F32, kind="ExternalInput")
    dis_d = nc.dram_tensor("dis", [128, bpc], F32, kind="ExternalInput")
    iota_d = nc.dram_tensor("iota", [128, 128], BF16, kind="ExternalInput")
    idx_d = nc.dram_tensor("idx", [128, TT * 8], I16, kind="ExternalInput")
    dstc_d = nc.dram_tensor("dstc", [128, TT * 2], F32, kind="ExternalInput")
    outT_d = nc.dram_tensor("outT", [4, npc], F32, kind="ExternalOutput")

    with tile.TileContext(nc) as tc, ExitStack() as ctx:
        dram = ctx.enter_context(tc.tile_pool(name="dram", bufs=1, space="DRAM"))
        const = ctx.enter_context(tc.tile_pool(name="const", bufs=1))
        xblk = ctx.enter_context(tc.tile_pool(name="xblk", bufs=4))
        ppsum = ctx.enter_context(tc.tile_pool(name="ppsum", bufs=2, space="PSUM"))
        pout = ctx.enter_context(tc.tile_pool(name="pout", bufs=4))
        idxp = ctx.enter_context(tc.tile_pool(name="idxp", bufs=3))
        gpool = ctx.enter_context(tc.tile_pool(name="gpool", bufs=2))
        ohp = ctx.enter_context(tc.tile_pool(name="ohp", bufs=4))
        apsum = ctx.enter_context(tc.tile_pool(name="apsum", bufs=4, space="PSUM"))
        zpool = ctx.enter_context(tc.tile_pool(name="zpool", bufs=1))
        hpool = ctx.enter_context(tc.tile_pool(name="hpool", bufs=1))
        opsum = ctx.enter_context(tc.tile_pool(name="opsum", bufs=2, space="PSUM"))
        outp = ctx.enter_context(tc.tile_pool(name="outp", bufs=4))

        p_bounce = dram.tile([npc, 128], BF16)
        p_table = dram.tile([cfg.npad, 128], BF16)
        g_bounce = dram.tile([npc, 128], BF16)
        g_table = dram.tile([cfg.npad, 128], BF16)

        nc.gpsimd.load_library(mlp)

        iota_t = const.tile([128, 128], BF16)
        nc.sync.dma_start(iota_t[:], iota_d[:, :])
        dis_t = const.tile([128, bpc], F32)
        nc.sync.dma_start(dis_t[:], dis_d[:, :])
        b1_t = const.tile([128, 1], F32)
        nc.sync.dma_start(b1_t[:], b1_d[:, :])
        b2_t = const.tile([128, 1], F32)
        nc.sync.dma_start(b2_t[:], b2_d[:, :])
        bfc_t = const.tile([4, 1], F32)
        nc.sync.dma_start(bfc_t[:], bfc_d[:, :])
        w1_t = const.tile([4, 128], F32)
        nc.sync.dma_start(w1_t[:], w1_d[:, :])
        w2f_t = const.tile([128, 128], F32)
        nc.sync.dma_start(w2f_t[:], w2_d[:, :])
        wfcf_t = const.tile([128, 4], F32)
        nc.sync.dma_start(wfcf_t[:], wfc_d[:, :])
        dstc_t = const.tile([128, TT * 2], F32)
        nc.sync.dma_start(dstc_t[:], dstc_d[:, :])

        w2b_t = const.tile([128, 128], BF16)
        nc.vector.tensor_copy(w2b_t[:], w2f_t[:])
        wfcb_t = const.tile([128, 4], BF16)
        nc.vector.tensor_copy(wfcb_t[:], wfcf_t[:])

        def table_build(hsrc, bounce, kind):
            # kind "p": lhsT = x block [4, blk] fp32, rhs = w1 [4,128] fp32
            # kind "g": lhsT = h1T slice [128, blk] bf16, rhs = w2 [128,128] bf16
            for b in range(bpc):
                sl = slice(b * blk, (b + 1) * blk)
                ps = ppsum.tile([128, 128], F32)
                if kind == "p":
                    xb = xblk.tile([4, blk], F32)
                    nc.sync.dma_start(xb[:], xT_d[:, sl])
                    nc.tensor.matmul(ps[:], xb[:], w1_t[:], start=True, stop=True)
                else:
                    nc.tensor.matmul(
                        ps[:], hsrc[:, sl], w2b_t[:], start=True, stop=True
                    )
                pb = pout.tile([128, 128], BF16)
                nc.vector.tensor_scalar(
                    pb[:], ps[:], dis_t[:, b : b + 1], None, mybir.AluOpType.mult
                )
                nc.sync.dma_start(bounce[sl, :], pb[:])

        def agg_layer(table, Z):
            for c in range(cfg.nchunks):
                tbl = table[c * cfg.chunk_rows : (c + 1) * cfg.chunk_rows, :]
                ps = None  # segment accumulator persists across call boundaries
                for gstart, tlist in meta[c]:
                    nt = len(tlist)
                    ni = nt * blk
                    it = idxp.tile([128, nt * 8], I16)
                    nc.sync.dma_start(
                        it[:], idx_d[:, gstart * 8 : gstart * 8 + nt * 8]
                    )
                    gt = gpool.tile([128, nt, 128], BF16)
                    nc.gpsimd.dma_gather(gt[:], tbl, it[:], ni, ni, 128)
                    for t, (b, first, last) in enumerate(tlist):
                        g = gstart + t
                        oh = ohp.tile([128, 128], BF16)
                        nc.vector.tensor_scalar(
                            oh[:],
                            iota_t[:],
                            dstc_t[:, 2 * g : 2 * g + 1],
                            dstc_t[:, 2 * g + 1 : 2 * g + 2],
                            mybir.AluOpType.is_equal,
                            mybir.AluOpType.mult,
                        )
                        if first:
                            ps = apsum.tile([128, 128], F32)
                        assert ps is not None
                        nc.tensor.matmul(
                            ps[:], gt[:, t, :], oh[:], start=first, stop=last
                        )
                        if last:
                            zsl = Z[:, b * blk : (b + 1) * blk]
                            if c == 0:
                                nc.vector.tensor_copy(zsl, ps[:])
                            else:
                                nc.vector.tensor_add(zsl, zsl, ps[:])

        # P0
        table_build(None, p_bounce, "p")
        nc.gpsimd.collective_compute(
            "AllGather",
            mybir.AluOpType.bypass,
            replica_groups=[list(range(cfg.nc))],
            ins=[p_bounce.opt()],
            outs=[p_table.opt()],
        )

        Z1 = zpool.tile([128, npc], F32, tag="Z")
        agg_layer(p_table, Z1)
        h1T = hpool.tile([128, npc], BF16, tag="hT")
        for b in range(bpc):
            sl = slice(b * blk, (b + 1) * blk)
            nc.scalar.activation(
                h1T[:, sl], Z1[:, sl], mybir.ActivationFunctionType.Relu,
                bias=b1_t[:, 0:1],
            )

        # P2
        table_build(h1T, g_bounce, "g")
        nc.gpsimd.collective_compute(
            "AllGather",
            mybir.AluOpType.bypass,
            replica_groups=[list(range(cfg.nc))],
            ins=[g_bounce.opt()],
            outs=[g_table.opt()],
        )

        Z2 = zpool.tile([128, npc], F32, tag="Z")
        agg_layer(g_table, Z2)
        h2T = hpool.tile([128, npc], BF16, tag="hT")
        for b in range(bpc):
            sl = slice(b * blk, (b + 1) * blk)
            nc.scalar.activation(
                h2T[:, sl], Z2[:, sl], mybir.ActivationFunctionType.Relu,
                bias=b2_t[:, 0:1],
            )

        # P4
        for b in range(bpc):
            sl = slice(b * blk, (b + 1) * blk)
            ps4 = opsum.tile([4, 128], F32)
            nc.tensor.matmul(ps4[:], wfcb_t[:], h2T[:, sl], start=True, stop=True)
            ot = outp.tile([4, 128], F32)
            nc.vector.tensor_scalar(
                ot[:], ps4[:], bfc_t[:, 0:1], None, mybir.AluOpType.add
            )
            nc.sync.dma_start(outT_d[:, sl], ot[:])

    nc.compile()
    return nc


_CACHE: dict = {}


def _get_program(cfg: Cfg, TT: int, meta):
    key = (cfg, TT, tuple((g, tuple(tl)) for calls in meta for g, tl in calls))
    if key not in _CACHE:
        _CACHE[key] = _build(cfg, TT, meta)
    return _CACHE[key]


def _make_in_maps(cfg: Cfg, x, W1, b1, W2, b2, Wfc, bfc, idx_wrapped, dstc, dis):
    n, npc = cfg.n, cfg.nodes_pc
    xT = np.zeros((4, cfg.npad), np.float32)
    xT[:3, :n] = np.asarray(x, np.float32).T
    w1p = np.zeros((4, 128), np.float32)
    w1p[:3] = np.asarray(W1, np.float32)
    wfcp = np.zeros((128, 4), np.float32)
    wfcp[:, :3] = np.asarray(Wfc, np.float32)
    bfcp = np.zeros((4, 1), np.float32)
    bfcp[:3, 0] = np.asarray(bfc, np.float32)
    iota = (
        np.broadcast_to(np.arange(128, dtype=np.float32), (128, 128))
        .astype(ml_dtypes.bfloat16)
        .copy()
    )
    in_maps = []
    for c in range(cfg.nc):
        nsl = slice(c * npc, (c + 1) * npc)
        in_maps.append(
            {
                "xT": xT[:, nsl].copy(),
                "w1": w1p,
                "b1": np.asarray(b1, np.float32).reshape(128, 1),
                "w2": np.asarray(W2, np.float32),
                "b2": np.asarray(b2, np.float32).reshape(128, 1),
                "wfc": wfcp,
                "bfc": bfcp,
                "dis": dis[nsl].reshape(cfg.bpc, 128).T.copy(),
                "iota": np.asarray(iota),
                "idx": idx_wrapped[c],
                "dstc": dstc[c],
            }
        )
    return in_maps


def kernel(x, edge_index, W1, b1, W2, b2, Wfc, bfc, _cfg: Cfg = None):
    cfg = _cfg or CFG
    T, TT, meta, idx_wrapped, dstc, dis = _prep(cfg, np.asarray(edge_index))
    nc = _get_program(cfg, TT, meta)
    in_maps = _make_in_maps(cfg, x, W1, b1, W2, b2, Wfc, bfc, idx_wrapped, dstc, dis)
    res = run_bass_kernel_spmd(nc, in_maps, core_ids=list(range(cfg.nc)))
    out = np.concatenate(
        [res.results[c]["outT"].T for c in range(cfg.nc)], axis=0
    )
    return np.ascontiguousarray(out[: cfg.n, :3]).astype(np.float32)
